# revision 6
# baseline (speedup 1.0000x reference)
"""Trainium2 Bass kernel for a dense transformer block (attention + LoRA +
MLP + proj), data-parallel over batch across 8 NeuronCores.

Contract: kernel(**inputs) takes the FULL unsharded inputs (numpy arrays,
keys as in reference.setup_inputs()) and returns the FULL [8, 512, 1024]
fp32 output.

Design (per core, one batch element):
  - LoRA is merged into the dense weights on the host (W_eff = W + A@B in
    fp32, the standard LoRA-merge deployment transform), so the device
    kernel is a plain transformer block: 384 GEMM matmuls + attention.
  - Everything flows channel-major: activations are [C, S] tiles with
    channels on SBUF partitions; weights are used in their natural
    [C_in, C_out] layout.  The qkv weight columns are host-permuted
    pair-major ([q_p | k_p] for head pair p) so the qk GEMM produces one
    head-pair's q and k chunks from one contiguous weight stripe.
  - Attention is software-pipelined at head-pair granularity.  Steady
    block p: [QK(p) chunks 0-1, both heads] [qk GEMM chunks for later
    pairs] [QK(p) chunks 2-3] [more GEMM] [PV(p-1)].  The two heads of a
    pair run QK as concurrent 64x128 row tiles (T0 = partitions 0-63,
    T8 = 64-127, distinct PSUM banks).  exp (ACT, scale=1/8 folded in)
    evacuates QK PSUM during the interleaved GEMM matmuls; PV lags one
    block so ACT (~4.6us/pair) always hides under PE (>=5.2us/block).
  - The key mask is folded into v (masked key rows of token-major v and
    of its appended ones-columns are zeroed), so exp is bias-free and the
    softmax denominator comes free as a ones-column in the PV matmul
    (M=65).
  - Normalization: each head's denominator row (PV PSUM partition 64) is
    staged to SBUF and DMA'd into an [8, 512] per-half tile; one DVE
    reciprocal per half, then a K=8 selection matmul broadcasts
    reciprocals to [128, 512] per chunk for the xou multiply.
  - MLP/proj are m-outer GEMMs in 2-chunk groups (one 2-bank PSUM tile
    per group) so group epilogues overlap the next group's matmuls.
  - GEMMs run in bf16; PSUM accumulation is fp32; the softmax reciprocal
    path stays f32r (= fp32 bits).
  - PSUM budget (8 banks): qk2 tag 2x2 + gchunk 2x1 + pv 2x1.
"""

import numpy as np

B, S, C = 8, 512, 1024
H, HD, R, HID = 16, 64, 32, 1024
NCORES = 8
KC = C // 128           # 8 contraction chunks
NPAIR = 8               # head pairs
VSTRIDE = HD + 1        # v columns per head incl. ones column

_cache = {}


def _get_nc():
    if "nc" in _cache:
        return _cache["nc"]

    from contextlib import ExitStack
    import concourse.tile as tile
    from concourse import bacc, mybir

    f32 = mybir.dt.float32
    f32r = mybir.dt.float32r
    bf16 = mybir.dt.bfloat16
    AF = mybir.ActivationFunctionType
    ALU = mybir.AluOpType

    nc = bacc.Bacc("TRN2", target_bir_lowering=False, debug=False)

    def din(name, shape, dt=bf16):
        return nc.dram_tensor(name, list(shape), dt, kind="ExternalInput")

    xT_d = din("xT", (C, S))
    mask01_d = din("mask01", (128, 4), f32)
    sel8_d = din("sel8", (8, 512), f32r)
    wqk_d = din("wqk", (C, 2 * C))      # pair-major permuted [q_p | k_p]
    wv_d = din("wv", (C, C))
    w1_d = din("w1", (C, HID))
    w2_d = din("w2", (HID, C))
    wp_d = din("wp", (C, C))
    b1_d = din("b1", (HID,), f32)
    b2_d = din("b2", (C,), f32)
    bp_d = din("bp", (C,), f32)
    outT_d = nc.dram_tensor("outT", [C, S], f32, kind="ExternalOutput")

    with tile.TileContext(nc) as tc, ExitStack() as ctx:
        resident = ctx.enter_context(tc.tile_pool(name="resident", bufs=1))
        wqkp = ctx.enter_context(tc.tile_pool(name="wqkp", bufs=32))
        pool2 = ctx.enter_context(tc.tile_pool(name="pool2", bufs=2, space="PSUM"))
        pool1 = ctx.enter_context(tc.tile_pool(name="pool1", bufs=2, space="PSUM"))
        expp = ctx.enter_context(tc.tile_pool(name="expp", bufs=4))
        tmpp = ctx.enter_context(tc.tile_pool(name="tmpp", bufs=2))
        tmbp = ctx.enter_context(tc.tile_pool(name="tmbp", bufs=2))
        outp = ctx.enter_context(tc.tile_pool(name="outp", bufs=2))

        # ---- resident loads (gpsimd queue) ---------------------------------
        xT = resident.tile([128, KC, S], bf16, name="xT", tag="xT")
        xT_r = xT_d[:].rearrange("(c p) s -> p c s", p=128)
        for kc in range(KC):
            nc.gpsimd.dma_start(xT[:, kc, :], xT_r[:, kc, :])
        mask01 = resident.tile([128, 4], f32, name="mask01", tag="mask01")
        nc.gpsimd.dma_start(mask01[:], mask01_d[:])
        sel8 = resident.tile([8, 512], f32r, name="sel8", tag="sel8")
        nc.gpsimd.dma_start(sel8[:], sel8_d[:])
        biases = {}
        for nm, b_d in (("fc1", b1_d), ("fc2", b2_d), ("proj", bp_d)):
            biases[nm] = resident.tile([128, KC], f32, name=f"b_{nm}",
                                       tag=f"b_{nm}")
            nc.gpsimd.dma_start(
                biases[nm][:], b_d[:].rearrange("(m p) -> p m", p=128)
            )

        # preload the exp activation table before the attention phase needs
        # it (first use would otherwise stall ~2.7us mid-pipeline)
        dummy = resident.tile([8, 4], f32, name="dummy", tag="dummy")
        nc.scalar.activation(dummy[:], sel8[:, 0:4], AF.Exp)

        wqk_r = wqk_d[:].rearrange("(k p) n -> k p n", p=128)
        wv_r = wv_d[:].rearrange("(k p) n -> k p n", p=128)
        w_mlp_r = {
            "fc1": w1_d[:].rearrange("(k p) n -> k p n", p=128),
            "fc2": w2_d[:].rearrange("(k p) n -> k p n", p=128),
            "proj": wp_d[:].rearrange("(k p) n -> k p n", p=128),
        }

        # wv and the MLP weights are resident: [128, 1024] tiles (2KB DMA
        # lines) fetched on the scalar/gpsimd DMA queues well before use.
        # (Each DMA has ~0.6us fixed cost, so few big transfers beat many
        # small ones; the DMA queues are hardware, independent of ACT work.)
        wv_t = resident.tile([128, KC, C], bf16, name="wv_t", tag="wv_t")
        for kc in range(KC):
            nc.scalar.dma_start(wv_t[:, kc, :], wv_r[kc, :, :])
        w_mlp_t = {}
        for nm, queue in (("fc1", nc.scalar), ("proj", nc.scalar),
                          ("fc2", nc.gpsimd)):
            w_mlp_t[nm] = resident.tile([128, KC, C], bf16, name=f"w_{nm}",
                                        tag=f"w_{nm}")
            for kc in range(KC):
                queue.dma_start(w_mlp_t[nm][:, kc, :], w_mlp_r[nm][kc, :, :])

        # ---- persistent activation tiles -----------------------------------
        # qkT: pair-major, [128, pair, {q,k}, S]
        qkT = resident.tile([128, NPAIR, 2, S], bf16, name="qkT", tag="qkT")
        v = resident.tile([128, 4, H * VSTRIDE], bf16, name="vtok", tag="vtok")
        xou = resident.tile([128, KC, S], bf16, name="xou", tag="xou")
        gT = resident.tile([128, KC, S], bf16, name="gT", tag="gT")
        xo2T = resident.tile([128, KC, S], bf16, name="xo2T", tag="xo2T")
        den128 = [
            resident.tile([128, 32], f32r, name=f"den128_{hb}",
                          tag=f"den128_{hb}")
            for hb in range(2)
        ]
        recip128 = [
            resident.tile([128, 32], f32r, name=f"recip128_{hb}",
                          tag=f"recip128_{hb}")
            for hb in range(2)
        ]
        recip8 = [
            resident.tile([8, S], f32r, name=f"recip8_{hb}", tag=f"recip8_{hb}")
            for hb in range(2)
        ]

        # ---- helpers --------------------------------------------------------
        def gemm_chunk(dst_ap, w_slices, act):
            """One [128, S] output chunk: 8 accumulating matmuls, CAST out."""
            pt = pool1.tile([128, S], f32, name="gchunk", tag="gchunk")
            for kc in range(KC):
                nc.tensor.matmul(
                    pt[:], w_slices[kc], act[:, kc, :],
                    start=(kc == 0), stop=(kc == KC - 1),
                )
            nc.vector.tensor_copy(dst_ap, pt[:])

        def qk_weights(p):
            """Stream pair p's weight tiles [128, 256] per kc (sync queue);
            returns (q_slices, k_slices)."""
            qs, ks = [], []
            for kc in range(KC):
                wt = wqkp.tile([128, 256], bf16, name=f"wqk{p}{kc}", tag="wqk")
                nc.sync.dma_start(wt[:], wqk_r[kc, :, p * 256:(p + 1) * 256])
                qs.append(wt[:, 0:128])
                ks.append(wt[:, 128:256])
            return qs, ks

        def qk_half(p, h, exp_tiles):
            """QK chunks 2h, 2h+1 for both heads of pair p (row tiles T0/T8),
            then exp into exp_tiles[e][:, 2h:2h+2, :]."""
            ts = [
                pool2.tile([128, 2, S], f32, name=f"qk{p}{h}{e}", tag="qk2")
                for e in range(2)
            ]
            for i in range(2):
                cb = (2 * h + i) * 128
                for e in range(2):
                    p0 = 64 * e
                    nc.tensor.matmul(
                        ts[e][:, i, :],
                        qkT[p0:p0 + 64, p, 1, cb:cb + 128],
                        qkT[p0:p0 + 64, p, 0, :],
                    )
            for e in range(2):
                nc.scalar.activation(
                    exp_tiles[e][:, 2 * h:2 * h + 2, :], ts[e][:],
                    AF.Exp, scale=0.125,
                )

        def pv_pair(p, exp_tiles):
            """PV for both heads of pair p; returns (pv_even, pv_odd)."""
            pvs = []
            for e in range(2):
                ph = 2 * p + e
                pv = pool1.tile([128, S], f32, name=f"pv{ph}", tag="pv")
                for c in range(4):
                    nc.tensor.matmul(
                        pv[0:VSTRIDE, :],
                        v[:, c, ph * VSTRIDE:(ph + 1) * VSTRIDE],
                        exp_tiles[e][:, c, :],
                        start=(c == 0), stop=(c == 3),
                    )
                pvs.append(pv)
            return pvs

        def finish_pair(p, pvs):
            """Evacuate xou chunks + denominator rows for pair p."""
            hb, row = p // 4, (2 * p) % 8
            stage = tmpp.tile([128, 2, S], f32r, name=f"st{p}", tag="stage")
            for e in range(2):
                nc.vector.tensor_copy(
                    stage[HD:HD + 1, e, :], pvs[e][HD:HD + 1, :]
                )
            # scatter the pair's two denominator rows into the [128, 32]
            # reciprocal layout (flat order: head-local*512 + query)
            nc.gpsimd.dma_start(den128[hb][row * 16:row * 16 + 32, :],
                                stage[HD:HD + 1, :, :])
            # even head -> xou rows 0-63 directly; odd head via bf16 staging
            # + DMA (cross-partition move)
            nc.vector.tensor_copy(xou[0:64, p, :], pvs[0][0:HD, :])
            tmb = tmbp.tile([128, S], bf16, name=f"tb{p}", tag="tmb")
            nc.vector.tensor_copy(tmb[0:HD, :], pvs[1][0:HD, :])
            nc.gpsimd.dma_start(xou[64:128, p, :], tmb[0:HD, :])

        def norm_half(hb):
            # reciprocal on all 128 partitions (DVE reciprocal is
            # partition-serial, ~6.5ns/elem), then DMA-gather to [8, 512]
            with nc.allow_low_precision(reason="f32r keeps fp32 bits"):
                nc.vector.reciprocal(recip128[hb][:], den128[hb][:])
            nc.gpsimd.dma_start(recip8[hb][:], recip128[hb][:])
            for jj in range(4):
                j = hb * 4 + jj
                pn = pool1.tile([128, S], f32, name=f"pn{j}", tag="gchunk")
                nc.tensor.matmul(
                    pn[:], sel8[:, jj * 128:(jj + 1) * 128], recip8[hb][:]
                )
                nc.vector.tensor_mul(xou[:, j, :], xou[:, j, :], pn[:])

        # ---- prologue -------------------------------------------------------
        for hh in range(H):
            nc.vector.memset(
                v[:, :, hh * VSTRIDE + HD:hh * VSTRIDE + HD + 1], 1.0
            )
        for cc in range(4):
            ones_cols = v[:, cc, :].rearrange("p (h z) -> p h z", z=VSTRIDE)[
                :, :, HD:HD + 1
            ]
            nc.vector.tensor_scalar_mul(
                ones_cols, ones_cols, mask01[:, cc:cc + 1]
            )

        def v_half(n):
            """v columns [n*512, (n+1)*512) -> heads 8n..8n+7, token-major."""
            pa = pool2.tile([128, 2, S], f32, name=f"va{n}", tag="qk2")
            pb = pool2.tile([128, 2, S], f32, name=f"vb{n}", tag="qk2")
            halves = (pa, pb)
            for kc in range(KC):
                for cc in range(4):
                    nc.tensor.matmul(
                        halves[cc // 2][:, cc % 2, :],
                        xT[:, kc, cc * 128:(cc + 1) * 128],
                        wv_t[:, kc, n * 512:(n + 1) * 512],
                        start=(kc == 0), stop=(kc == KC - 1),
                    )
            for cc in range(4):
                pm = halves[cc // 2][:, cc % 2, :]
                dst = v[:, cc, n * 8 * VSTRIDE:(n + 1) * 8 * VSTRIDE].rearrange(
                    "p (h z) -> p h z", z=VSTRIDE
                )[:, :, 0:HD]
                src = pm.rearrange("p (h z) -> p h z", z=HD)
                nc.vector.tensor_scalar_mul(dst, src, mask01[:, cc:cc + 1])

        # v half 0 (needed by PV(0)), then pairs 0/1 qk chunks, then v half 1
        pair_w = {}
        v_half(0)
        for p in (0, 1):
            pair_w[p] = qk_weights(p)
            gemm_chunk(qkT[:, p, 0, :], pair_w[p][0], xT)
            gemm_chunk(qkT[:, p, 1, :], pair_w[p][1], xT)
        for g in (2, 3):
            pair_w[g] = qk_weights(g)
        v_half(1)

        # ---- steady attention pipeline --------------------------------------
        # Remaining qk GEMM chunks (pairs 2-7 = 12 chunks) are distributed
        # across blocks 0-7 as [2,2,2,2,1,1,1,1] so every block has enough
        # PE work to hide its pair's exp ACTs.  Chunk A runs before QK so
        # block 7's k7 lands before QK(7) consumes it; chunk B sits between
        # the QK halves so exp has time to free the first half's PSUM.
        chunks = []             # (pair, qk) in consumption order
        for g in range(2, NPAIR):
            chunks.append((g, 0))
            chunks.append((g, 1))
        block_sched = [[chunks[2 * p], chunks[2 * p + 1]] for p in range(4)]
        block_sched += [[chunks[8 + i], None] for i in range(4)]
        exp_prev = None
        for p in range(NPAIR):
            exp_tiles = [
                expp.tile([128, 4, S], bf16, name=f"exp{2 * p + e}", tag="exp")
                for e in range(2)
            ]
            slot_a, slot_b = block_sched[p]
            if p + 4 < NPAIR and (p + 4) not in pair_w:
                pair_w[p + 4] = qk_weights(p + 4)

            def do_chunk(slot):
                g, qk = slot
                gemm_chunk(qkT[:, g, qk, :], pair_w[g][qk], xT)

            do_chunk(slot_a)
            qk_half(p, 0, exp_tiles)
            if slot_b is not None:
                do_chunk(slot_b)
            qk_half(p, 1, exp_tiles)
            if exp_prev is not None:
                pvs = pv_pair(p - 1, exp_prev)
                finish_pair(p - 1, pvs)
                if p - 1 == 3:
                    norm_half(0)
            exp_prev = exp_tiles
        pvs = pv_pair(NPAIR - 1, exp_prev)
        finish_pair(NPAIR - 1, pvs)
        norm_half(1)

        # ---- MLP fc1 + gelu, fc2 + residual, proj ---------------------------
        def mlp_gemm(nm, act, epilogue):
            wt = w_mlp_t[nm]
            for g in range(4):
                pt2 = pool2.tile([128, 2, S], f32, name=f"p{nm}{g}", tag="qk2")
                for kc in range(KC):
                    for i in range(2):
                        m = 2 * g + i
                        nc.tensor.matmul(
                            pt2[:, i, :],
                            wt[:, kc, m * 128:(m + 1) * 128],
                            act[:, kc, :],
                            start=(kc == 0), stop=(kc == KC - 1),
                        )
                for i in range(2):
                    epilogue(2 * g + i, pt2[:, i, :])

        def fc1_epi(m, pm):
            nc.scalar.activation(
                gT[:, m, :], pm, AF.Gelu, bias=biases["fc1"][:, m:m + 1]
            )

        mlp_gemm("fc1", xou, fc1_epi)

        def fc2_epi(m, pm):
            nc.vector.scalar_tensor_tensor(
                xo2T[:, m, :], pm, biases["fc2"][:, m:m + 1],
                xou[:, m, :], op0=ALU.add, op1=ALU.add,
            )

        mlp_gemm("fc2", gT, fc2_epi)

        outT_r = outT_d[:].rearrange("(m p) s -> p m s", p=128)

        def proj_epi(m, pm):
            ot = outp.tile([128, S], f32, name=f"ot{m}", tag="out")
            nc.scalar.activation(
                ot[:], pm, AF.Identity, bias=biases["proj"][:, m:m + 1]
            )
            nc.sync.dma_start(outT_r[:, m, :], ot[:])

        mlp_gemm("proj", xo2T, proj_epi)

    nc.compile()
    _cache["nc"] = nc
    return nc


def _bf16(a):
    import ml_dtypes

    return np.asarray(a, dtype=np.float32).astype(ml_dtypes.bfloat16)


def _make_in_maps(inputs):
    x = np.asarray(inputs["x"], dtype=np.float32)
    mask = np.asarray(inputs["mask"])

    f = lambda k: np.asarray(inputs[k], dtype=np.float32)
    # LoRA merge (exact in fp32): W_eff = W + A @ B
    wqkv = f("qkv_w") + f("qkv_la") @ f("qkv_lb")
    w1 = f("fc1_w") + f("fc1_la") @ f("fc1_lb")
    w2 = f("fc2_w") + f("fc2_la") @ f("fc2_lb")
    wp = f("proj_w") + f("proj_la") @ f("proj_lb")

    # pair-major qk permutation: [q_p (128 cols) | k_p (128 cols)] per pair
    wqk = np.empty((C, 2 * C), dtype=np.float32)
    for p in range(NPAIR):
        wqk[:, p * 256:p * 256 + 128] = wqkv[:, p * 128:(p + 1) * 128]
        wqk[:, p * 256 + 128:p * 256 + 256] = \
            wqkv[:, C + p * 128:C + (p + 1) * 128]
    wv = wqkv[:, 2 * C:]

    sel8 = np.zeros((8, 512), dtype=np.float32)
    for jj in range(4):
        for p in range(128):
            sel8[2 * jj + p // 64, jj * 128 + p] = 1.0

    shared = {
        "sel8": sel8,
        "wqk": np.ascontiguousarray(_bf16(wqk)),
        "wv": np.ascontiguousarray(_bf16(wv)),
        "w1": np.ascontiguousarray(_bf16(w1)),
        "w2": np.ascontiguousarray(_bf16(w2)),
        "wp": np.ascontiguousarray(_bf16(wp)),
        "b1": np.ascontiguousarray(inputs["fc1_b"], dtype=np.float32),
        "b2": np.ascontiguousarray(inputs["fc2_b"], dtype=np.float32),
        "bp": np.ascontiguousarray(inputs["proj_b"], dtype=np.float32),
    }
    in_maps = []
    for b in range(NCORES):
        m01 = mask[b, :S].astype(np.float32)          # 1.0 keep / 0.0 drop
        in_maps.append(
            dict(
                shared,
                xT=np.ascontiguousarray(_bf16(x[b].T)),
                mask01=np.ascontiguousarray(m01.reshape(4, 128).T),
            )
        )
    return in_maps


def _run(inputs, trace=False):
    from concourse.bass_utils import run_bass_kernel_spmd

    nc = _get_nc()
    in_maps = _make_in_maps(inputs)
    res = run_bass_kernel_spmd(nc, in_maps, list(range(NCORES)), trace=trace)
    out = np.stack(
        [np.ascontiguousarray(res.results[b]["outT"].T) for b in range(NCORES)]
    )
    return out, res


def kernel(**inputs):
    out, _ = _run(inputs, trace=False)
    return out


# revision 7
# speedup vs baseline: 1.0771x; 1.0771x over previous
"""Trainium2 Bass kernel for a dense transformer block (attention + LoRA +
MLP + proj), data-parallel over batch across 8 NeuronCores.

Contract: kernel(**inputs) takes the FULL unsharded inputs (numpy arrays,
keys as in reference.setup_inputs()) and returns the FULL [8, 512, 1024]
fp32 output.

Design (per core, one batch element):
  - LoRA is merged into the dense weights on the host (W_eff = W + A@B in
    fp32, the standard LoRA-merge deployment transform), so the device
    kernel is a plain transformer block: 384 GEMM matmuls + attention.
  - Everything flows channel-major: activations are [C, S] tiles with
    channels on SBUF partitions; weights are used in their natural
    [C_in, C_out] layout.  The qkv weight columns are host-permuted
    pair-major ([q_p | k_p] for head pair p) so the qk GEMM produces one
    head-pair's q and k chunks from one contiguous weight stripe.
  - Attention is software-pipelined at head-pair granularity.  Steady
    block p: [QK(p) chunks 0-1, both heads] [qk GEMM chunks for later
    pairs] [QK(p) chunks 2-3] [more GEMM] [PV(p-1)].  The two heads of a
    pair run QK as concurrent 64x128 row tiles (T0 = partitions 0-63,
    T8 = 64-127, distinct PSUM banks).  exp (ACT, scale=1/8 folded in)
    evacuates QK PSUM during the interleaved GEMM matmuls; PV lags one
    block so ACT (~4.6us/pair) always hides under PE (>=5.2us/block).
  - The key mask is folded into v (masked key rows of token-major v and
    of its appended ones-columns are zeroed), so exp is bias-free and the
    softmax denominator comes free as a ones-column in the PV matmul
    (M=65).
  - Normalization: each head's denominator row (PV PSUM partition 64) is
    staged to SBUF and DMA'd into an [8, 512] per-half tile; one DVE
    reciprocal per half, then a K=8 selection matmul broadcasts
    reciprocals to [128, 512] per chunk for the xou multiply.
  - MLP/proj are m-outer GEMMs in 2-chunk groups (one 2-bank PSUM tile
    per group) so group epilogues overlap the next group's matmuls.
  - GEMMs run in bf16; PSUM accumulation is fp32; the softmax reciprocal
    path stays f32r (= fp32 bits).
  - PSUM budget (8 banks): qk2 tag 2x2 + gchunk 2x1 + pv 2x1.
"""

import numpy as np

B, S, C = 8, 512, 1024
H, HD, R, HID = 16, 64, 32, 1024
NCORES = 8
KC = C // 128           # 8 contraction chunks
NPAIR = 8               # head pairs
VSTRIDE = HD + 1        # v columns per head incl. ones column

_cache = {}


def _get_nc():
    if "nc" in _cache:
        return _cache["nc"]

    from contextlib import ExitStack
    import concourse.tile as tile
    from concourse import bacc, mybir

    f32 = mybir.dt.float32
    f32r = mybir.dt.float32r
    bf16 = mybir.dt.bfloat16
    AF = mybir.ActivationFunctionType
    ALU = mybir.AluOpType

    nc = bacc.Bacc("TRN2", target_bir_lowering=False, debug=False)

    def din(name, shape, dt=bf16):
        return nc.dram_tensor(name, list(shape), dt, kind="ExternalInput")

    xT_d = din("xT", (C, S))
    mask01_d = din("mask01", (128, 4), f32)
    sel8_d = din("sel8", (8, 512), f32r)
    wqk_d = din("wqk", (C, 2 * C))      # pair-major permuted [q_p | k_p]
    wv_d = din("wv", (C, C))
    w1_d = din("w1", (C, HID))
    w2_d = din("w2", (HID, C))
    wp_d = din("wp", (C, C))
    b1_d = din("b1", (HID,), f32)
    b2_d = din("b2", (C,), f32)
    bp_d = din("bp", (C,), f32)
    outT_d = nc.dram_tensor("outT", [C, S], f32, kind="ExternalOutput")

    with tile.TileContext(nc) as tc, ExitStack() as ctx:
        resident = ctx.enter_context(tc.tile_pool(name="resident", bufs=1))
        wqkp = ctx.enter_context(tc.tile_pool(name="wqkp", bufs=32))
        pool2 = ctx.enter_context(tc.tile_pool(name="pool2", bufs=2, space="PSUM"))
        pool1 = ctx.enter_context(tc.tile_pool(name="pool1", bufs=2, space="PSUM"))
        expp = ctx.enter_context(tc.tile_pool(name="expp", bufs=4))
        tmpp = ctx.enter_context(tc.tile_pool(name="tmpp", bufs=2))
        tmbp = ctx.enter_context(tc.tile_pool(name="tmbp", bufs=2))
        outp = ctx.enter_context(tc.tile_pool(name="outp", bufs=2))

        # ---- resident loads (gpsimd queue) ---------------------------------
        xT = resident.tile([128, KC, S], bf16, name="xT", tag="xT")
        xT_r = xT_d[:].rearrange("(c p) s -> p c s", p=128)
        for kc in range(KC):
            nc.gpsimd.dma_start(xT[:, kc, :], xT_r[:, kc, :])
        mask01 = resident.tile([128, 4], f32, name="mask01", tag="mask01")
        nc.gpsimd.dma_start(mask01[:], mask01_d[:])
        sel8 = resident.tile([8, 512], f32r, name="sel8", tag="sel8")
        nc.gpsimd.dma_start(sel8[:], sel8_d[:])
        biases = {}
        for nm, b_d in (("fc1", b1_d), ("fc2", b2_d), ("proj", bp_d)):
            biases[nm] = resident.tile([128, KC], f32, name=f"b_{nm}",
                                       tag=f"b_{nm}")
            nc.gpsimd.dma_start(
                biases[nm][:], b_d[:].rearrange("(m p) -> p m", p=128)
            )

        # preload the exp activation table before the attention phase needs
        # it (first use would otherwise stall ~2.7us mid-pipeline)
        dummy = resident.tile([8, 4], f32, name="dummy", tag="dummy")
        nc.scalar.activation(dummy[:], sel8[:, 0:4], AF.Exp)

        wqk_r = wqk_d[:].rearrange("(k p) n -> k p n", p=128)
        wv_r = wv_d[:].rearrange("(k p) n -> k p n", p=128)
        w_mlp_r = {
            "fc1": w1_d[:].rearrange("(k p) n -> k p n", p=128),
            "fc2": w2_d[:].rearrange("(k p) n -> k p n", p=128),
            "proj": wp_d[:].rearrange("(k p) n -> k p n", p=128),
        }

        # wv and the MLP weights are resident [128, 1024] tiles (2KB DMA
        # lines; each DMA has ~0.6us fixed cost so few big transfers win).
        # wv loads right after xT on gpsimd; MLP weights stream on the sync
        # queue 3-per-block during attention (never on the scalar engine --
        # a DMA trigger blocks the engine until the queue accepts it, which
        # would delay the exp ACTIVATEs).
        wv_t = resident.tile([128, KC, C], bf16, name="wv_t", tag="wv_t")
        for kc in range(KC):
            nc.gpsimd.dma_start(wv_t[:, kc, :], wv_r[kc, :, :])
        w_mlp_t = {}
        mlp_fetches = []
        for nm in ("fc1", "fc2", "proj"):
            w_mlp_t[nm] = resident.tile([128, KC, C], bf16, name=f"w_{nm}",
                                        tag=f"w_{nm}")
            for kc in range(KC):
                mlp_fetches.append((w_mlp_t[nm][:, kc, :],
                                    w_mlp_r[nm][kc, :, :]))

        # ---- persistent activation tiles -----------------------------------
        # qkT: pair-major, [128, pair, {q,k}, S]
        qkT = resident.tile([128, NPAIR, 2, S], bf16, name="qkT", tag="qkT")
        v = resident.tile([128, 4, H * VSTRIDE], bf16, name="vtok", tag="vtok")
        xou = resident.tile([128, KC, S], bf16, name="xou", tag="xou")
        gT = resident.tile([128, KC, S], bf16, name="gT", tag="gT")
        xo2T = resident.tile([128, KC, S], bf16, name="xo2T", tag="xo2T")
        den128 = [
            resident.tile([128, 32], f32r, name=f"den128_{hb}",
                          tag=f"den128_{hb}")
            for hb in range(2)
        ]
        recip128 = [
            resident.tile([128, 32], f32r, name=f"recip128_{hb}",
                          tag=f"recip128_{hb}")
            for hb in range(2)
        ]
        recip8 = [
            resident.tile([8, S], f32r, name=f"recip8_{hb}", tag=f"recip8_{hb}")
            for hb in range(2)
        ]

        # ---- helpers --------------------------------------------------------
        def gemm_chunk(dst_ap, w_slices, act):
            """One [128, S] output chunk: 8 accumulating matmuls, CAST out."""
            pt = pool1.tile([128, S], f32, name="gchunk", tag="gchunk")
            for kc in range(KC):
                nc.tensor.matmul(
                    pt[:], w_slices[kc], act[:, kc, :],
                    start=(kc == 0), stop=(kc == KC - 1),
                )
            nc.vector.tensor_copy(dst_ap, pt[:])

        def qk_weights(p):
            """Stream pair p's weight tiles [128, 256] per kc (sync queue);
            returns (q_slices, k_slices)."""
            qs, ks = [], []
            for kc in range(KC):
                wt = wqkp.tile([128, 256], bf16, name=f"wqk{p}{kc}", tag="wqk")
                nc.sync.dma_start(wt[:], wqk_r[kc, :, p * 256:(p + 1) * 256])
                qs.append(wt[:, 0:128])
                ks.append(wt[:, 128:256])
            return qs, ks

        def qk_half(p, h, exp_tiles):
            """QK chunks 2h, 2h+1 for both heads of pair p (row tiles T0/T8),
            then exp into exp_tiles[e][:, 2h:2h+2, :]."""
            ts = [
                pool2.tile([128, 2, S], f32, name=f"qk{p}{h}{e}", tag="qk2")
                for e in range(2)
            ]
            for i in range(2):
                cb = (2 * h + i) * 128
                for e in range(2):
                    p0 = 64 * e
                    nc.tensor.matmul(
                        ts[e][:, i, :],
                        qkT[p0:p0 + 64, p, 1, cb:cb + 128],
                        qkT[p0:p0 + 64, p, 0, :],
                    )
            for e in range(2):
                nc.scalar.activation(
                    exp_tiles[e][:, 2 * h:2 * h + 2, :], ts[e][:],
                    AF.Exp, scale=0.125,
                )

        def pv_pair(p, exp_tiles):
            """PV for both heads of pair p; returns (pv_even, pv_odd)."""
            pvs = []
            for e in range(2):
                ph = 2 * p + e
                pv = pool1.tile([128, S], f32, name=f"pv{ph}", tag="pv")
                for c in range(4):
                    nc.tensor.matmul(
                        pv[0:VSTRIDE, :],
                        v[:, c, ph * VSTRIDE:(ph + 1) * VSTRIDE],
                        exp_tiles[e][:, c, :],
                        start=(c == 0), stop=(c == 3),
                    )
                pvs.append(pv)
            return pvs

        def finish_pair(p, pvs):
            """Evacuate xou chunks + denominator rows for pair p."""
            hb, row = p // 4, (2 * p) % 8
            stage = tmpp.tile([128, 2, S], f32r, name=f"st{p}", tag="stage")
            for e in range(2):
                nc.vector.tensor_copy(
                    stage[HD:HD + 1, e, :], pvs[e][HD:HD + 1, :]
                )
            # scatter the pair's two denominator rows into the [128, 32]
            # reciprocal layout (flat order: head-local*512 + query)
            nc.gpsimd.dma_start(den128[hb][row * 16:row * 16 + 32, :],
                                stage[HD:HD + 1, :, :])
            # even head -> xou rows 0-63 directly; odd head via bf16 staging
            # + DMA (cross-partition move)
            nc.vector.tensor_copy(xou[0:64, p, :], pvs[0][0:HD, :])
            tmb = tmbp.tile([128, S], bf16, name=f"tb{p}", tag="tmb")
            nc.vector.tensor_copy(tmb[0:HD, :], pvs[1][0:HD, :])
            nc.gpsimd.dma_start(xou[64:128, p, :], tmb[0:HD, :])

        def norm_half(hb):
            # reciprocal on all 128 partitions (DVE reciprocal is
            # partition-serial, ~6.5ns/elem), then DMA-gather to [8, 512]
            with nc.allow_low_precision(reason="f32r keeps fp32 bits"):
                nc.vector.reciprocal(recip128[hb][:], den128[hb][:])
            nc.gpsimd.dma_start(recip8[hb][:], recip128[hb][:])
            for jj in range(4):
                j = hb * 4 + jj
                pn = pool1.tile([128, S], f32, name=f"pn{j}", tag="gchunk")
                nc.tensor.matmul(
                    pn[:], sel8[:, jj * 128:(jj + 1) * 128], recip8[hb][:]
                )
                nc.vector.tensor_mul(xou[:, j, :], xou[:, j, :], pn[:])

        # ---- prologue -------------------------------------------------------
        for hh in range(H):
            nc.vector.memset(
                v[:, :, hh * VSTRIDE + HD:hh * VSTRIDE + HD + 1], 1.0
            )
        for cc in range(4):
            ones_cols = v[:, cc, :].rearrange("p (h z) -> p h z", z=VSTRIDE)[
                :, :, HD:HD + 1
            ]
            nc.vector.tensor_scalar_mul(
                ones_cols, ones_cols, mask01[:, cc:cc + 1]
            )

        def v_half(n):
            """v columns [n*512, (n+1)*512) -> heads 8n..8n+7, token-major."""
            pa = pool2.tile([128, 2, S], f32, name=f"va{n}", tag="qk2")
            pb = pool2.tile([128, 2, S], f32, name=f"vb{n}", tag="qk2")
            halves = (pa, pb)
            for kc in range(KC):
                for cc in range(4):
                    nc.tensor.matmul(
                        halves[cc // 2][:, cc % 2, :],
                        xT[:, kc, cc * 128:(cc + 1) * 128],
                        wv_t[:, kc, n * 512:(n + 1) * 512],
                        start=(kc == 0), stop=(kc == KC - 1),
                    )
            for cc in range(4):
                pm = halves[cc // 2][:, cc % 2, :]
                dst = v[:, cc, n * 8 * VSTRIDE:(n + 1) * 8 * VSTRIDE].rearrange(
                    "p (h z) -> p h z", z=VSTRIDE
                )[:, :, 0:HD]
                src = pm.rearrange("p (h z) -> p h z", z=HD)
                nc.vector.tensor_scalar_mul(dst, src, mask01[:, cc:cc + 1])

        # Front: qk pair 0 (sync stream) interleaved with v half 0 (gpsimd
        # stream) so both DMA queues feed the PE concurrently.  v half 1 is
        # only needed by PV(4) and runs between blocks 0 and 1.
        pair_w = {}
        pair_w[0] = qk_weights(0)
        gemm_chunk(qkT[:, 0, 0, :], pair_w[0][0], xT)
        gemm_chunk(qkT[:, 0, 1, :], pair_w[0][1], xT)
        v_half(0)
        pair_w[1] = qk_weights(1)
        gemm_chunk(qkT[:, 1, 0, :], pair_w[1][0], xT)
        gemm_chunk(qkT[:, 1, 1, :], pair_w[1][1], xT)
        for g in (2, 3):
            pair_w[g] = qk_weights(g)

        # ---- steady attention pipeline --------------------------------------
        # Remaining qk GEMM chunks (pairs 2-7 = 12 chunks) are distributed
        # across blocks 0-7 as [2,2,2,2,1,1,1,1] so every block has enough
        # PE work to hide its pair's exp ACTs.  Chunk A runs before QK so
        # block 7's k7 lands before QK(7) consumes it; chunk B sits between
        # the QK halves so exp has time to free the first half's PSUM.
        chunks = []             # (pair, qk) in consumption order
        for g in range(2, NPAIR):
            chunks.append((g, 0))
            chunks.append((g, 1))
        block_sched = [[chunks[2 * p], chunks[2 * p + 1]] for p in range(4)]
        block_sched += [[chunks[8 + i], None] for i in range(4)]
        exp_prev = None
        for p in range(NPAIR):
            exp_tiles = [
                expp.tile([128, 4, S], bf16, name=f"exp{2 * p + e}", tag="exp")
                for e in range(2)
            ]
            slot_a, slot_b = block_sched[p]
            if p == 1:
                v_half(1)
            if p + 4 < NPAIR and (p + 4) not in pair_w:
                pair_w[p + 4] = qk_weights(p + 4)
            for _ in range(3):
                if mlp_fetches:
                    dst, srcap = mlp_fetches.pop(0)
                    nc.sync.dma_start(dst, srcap)

            def do_chunk(slot):
                g, qk = slot
                gemm_chunk(qkT[:, g, qk, :], pair_w[g][qk], xT)

            do_chunk(slot_a)
            qk_half(p, 0, exp_tiles)
            if slot_b is not None:
                do_chunk(slot_b)
            qk_half(p, 1, exp_tiles)
            if exp_prev is not None:
                pvs = pv_pair(p - 1, exp_prev)
                finish_pair(p - 1, pvs)
                if p - 1 == 3:
                    norm_half(0)
            exp_prev = exp_tiles
        pvs = pv_pair(NPAIR - 1, exp_prev)
        finish_pair(NPAIR - 1, pvs)
        norm_half(1)

        # ---- MLP fc1 + gelu, fc2 + residual, proj ---------------------------
        def mlp_gemm(nm, act, epilogue):
            wt = w_mlp_t[nm]
            for g in range(4):
                pt2 = pool2.tile([128, 2, S], f32, name=f"p{nm}{g}", tag="qk2")
                for kc in range(KC):
                    for i in range(2):
                        m = 2 * g + i
                        nc.tensor.matmul(
                            pt2[:, i, :],
                            wt[:, kc, m * 128:(m + 1) * 128],
                            act[:, kc, :],
                            start=(kc == 0), stop=(kc == KC - 1),
                        )
                for i in range(2):
                    epilogue(2 * g + i, pt2[:, i, :])

        def fc1_epi(m, pm):
            nc.scalar.activation(
                gT[:, m, :], pm, AF.Gelu, bias=biases["fc1"][:, m:m + 1]
            )

        mlp_gemm("fc1", xou, fc1_epi)

        def fc2_epi(m, pm):
            nc.vector.scalar_tensor_tensor(
                xo2T[:, m, :], pm, biases["fc2"][:, m:m + 1],
                xou[:, m, :], op0=ALU.add, op1=ALU.add,
            )

        mlp_gemm("fc2", gT, fc2_epi)

        outT_r = outT_d[:].rearrange("(m p) s -> p m s", p=128)

        def proj_epi(m, pm):
            ot = outp.tile([128, S], f32, name=f"ot{m}", tag="out")
            nc.scalar.activation(
                ot[:], pm, AF.Identity, bias=biases["proj"][:, m:m + 1]
            )
            nc.sync.dma_start(outT_r[:, m, :], ot[:])

        mlp_gemm("proj", xo2T, proj_epi)

    nc.compile()
    _cache["nc"] = nc
    return nc


def _bf16(a):
    import ml_dtypes

    return np.asarray(a, dtype=np.float32).astype(ml_dtypes.bfloat16)


def _make_in_maps(inputs):
    x = np.asarray(inputs["x"], dtype=np.float32)
    mask = np.asarray(inputs["mask"])

    f = lambda k: np.asarray(inputs[k], dtype=np.float32)
    # LoRA merge (exact in fp32): W_eff = W + A @ B
    wqkv = f("qkv_w") + f("qkv_la") @ f("qkv_lb")
    w1 = f("fc1_w") + f("fc1_la") @ f("fc1_lb")
    w2 = f("fc2_w") + f("fc2_la") @ f("fc2_lb")
    wp = f("proj_w") + f("proj_la") @ f("proj_lb")

    # pair-major qk permutation: [q_p (128 cols) | k_p (128 cols)] per pair
    wqk = np.empty((C, 2 * C), dtype=np.float32)
    for p in range(NPAIR):
        wqk[:, p * 256:p * 256 + 128] = wqkv[:, p * 128:(p + 1) * 128]
        wqk[:, p * 256 + 128:p * 256 + 256] = \
            wqkv[:, C + p * 128:C + (p + 1) * 128]
    wv = wqkv[:, 2 * C:]

    sel8 = np.zeros((8, 512), dtype=np.float32)
    for jj in range(4):
        for p in range(128):
            sel8[2 * jj + p // 64, jj * 128 + p] = 1.0

    shared = {
        "sel8": sel8,
        "wqk": np.ascontiguousarray(_bf16(wqk)),
        "wv": np.ascontiguousarray(_bf16(wv)),
        "w1": np.ascontiguousarray(_bf16(w1)),
        "w2": np.ascontiguousarray(_bf16(w2)),
        "wp": np.ascontiguousarray(_bf16(wp)),
        "b1": np.ascontiguousarray(inputs["fc1_b"], dtype=np.float32),
        "b2": np.ascontiguousarray(inputs["fc2_b"], dtype=np.float32),
        "bp": np.ascontiguousarray(inputs["proj_b"], dtype=np.float32),
    }
    in_maps = []
    for b in range(NCORES):
        m01 = mask[b, :S].astype(np.float32)          # 1.0 keep / 0.0 drop
        in_maps.append(
            dict(
                shared,
                xT=np.ascontiguousarray(_bf16(x[b].T)),
                mask01=np.ascontiguousarray(m01.reshape(4, 128).T),
            )
        )
    return in_maps


def _run(inputs, trace=False):
    from concourse.bass_utils import run_bass_kernel_spmd

    nc = _get_nc()
    in_maps = _make_in_maps(inputs)
    res = run_bass_kernel_spmd(nc, in_maps, list(range(NCORES)), trace=trace)
    out = np.stack(
        [np.ascontiguousarray(res.results[b]["outT"].T) for b in range(NCORES)]
    )
    return out, res


def kernel(**inputs):
    out, _ = _run(inputs, trace=False)
    return out
